# revision 58
# baseline (speedup 1.0000x reference)
"""MoE gate Trainium2 kernel, v4: fp32r main GEMM + fp8e5m2 DoubleRow
correction pass. Exact routing (idx matches fp32 reference bit-for-bit on
the eval data; weight rel err ~1e-5) at ~1/4 the PE cost of plain fp32.

Precision scheme (hardware-validated):
  - fp32r matmuls (tf32-like: operands rounded to 11 mantissa bits, 1
    cycle/row at free-dim >= 256) alone flip ~40 near-tie tokens.
  - logits = h1@w1 + (h2@w1s + (h1*2^-6)@(w2s*2^6)) where h1 = f32r(h),
    h2 = h - h1, w1 = f32r(w), w2 = w - w1. The two correction terms are
    e5m2 fp8 at matched true scale, packed as the two "rows" of ONE
    DoubleRow matmul group per K-tile (d = w[0]*m[0] + w[1]*m[1]), so they
    accumulate straight into the main PSUM tile at 0.5 cycles/row.
  - e5m2/DR numerics match ml_dtypes simulation exactly; residual error
    ~1e-5 relative, zero top-k flips with margin.

Per core (2048 tokens), per 1024-token half, per K-tile (128 of 7168):
  - natural h loads [128t, 896] (8 chunks, 16-buf prefetch window)
  - 8 PE fp32 transposes into PSUM [128k, 1024t]
  - evacuation fans out to two engines: ACT rounds psum -> h1T (f32r),
    and ACT scale-copies h1T*2^-6 -> fp8 half of hc; DVE subtracts
    (psum - h1T) -> fp8 half of hc. (Pool/gpsimd is a slow DSP:
    ~15 cyc/elem — never give it bulk work.)
  - GEMM trails one K-tile (main f32r, 4x [128e,512t]) / two K-tiles
    (correction DR group, 4x [128e,512t] at half rate), accumulating over
    all 56 K-tiles into 4 PSUM banks.
  - logits^T transposed back on PE to [128t, 256e]; sigmoid (ACT) +
    top-k routing (DVE max8/max_index/match_replace; tiny ops on Pool).

w1 (f32r bits) and the packed fp8 correction weights are prepared on the
host (w is tiny and replicated) and DMA'd once as resident tiles.
"""

import numpy as np

import concourse.bass as bass
import concourse.mybir as mybir
import concourse.tile as tile
from concourse import bacc
from concourse.bass_utils import run_bass_kernel_spmd
from concourse.masks import make_identity

N_CORES = 8
T_FULL = 16384
H = 7168
E = 256
TOP_K = 8
N_GROUP = 8
PER_GROUP = E // N_GROUP
ROUTED_SCALING = 2.5

T_CORE = T_FULL // N_CORES  # 2048
KT = H // 128  # 56
HALF = 1024
N_HALF = T_CORE // HALF  # 2
TT_HALF = HALF // 128  # 8
KC = 8  # K chunks
KPC = KT // KC  # 7 K-tiles per chunk

F32 = mybir.dt.float32
F32R = mybir.dt.float32r
F8E5 = mybir.dt.float8e5
U32 = mybir.dt.uint32
I32 = mybir.dt.int32
BIG = 1.0e9
NPAIR = KT // 2  # 28 DoubleRow k-tile pairs

LAST_EXEC_NS = None


def rnd11(x):
    """Round fp32 to 11 mantissa bits (fp32r), round-to-nearest-even."""
    x = np.ascontiguousarray(x, dtype=np.float32)
    xi = x.view(np.uint32).astype(np.uint64)
    xi = (xi + np.uint64(1 << 11)) & np.uint64(0xFFFFF000)
    return (xi & np.uint64(0xFFFFFFFF)).astype(np.uint32).view(np.float32)


def _e5m2(x):
    import ml_dtypes
    return np.asarray(x, np.float32).astype(ml_dtypes.float8_e5m2)


def dr_pack_terms(wa, wb):
    """Two [H, E] fp8 arrays -> per-k-tile DoubleRow term layout
    [KT*128, 2*E]: row (k*128+p), col (i*E+e) = (wa if i==0 else wb)[k*128+p, e].
    The DR 'pair' dim carries the two correction terms."""
    a = np.stack([wa.reshape(KT, 128, E), wb.reshape(KT, 128, E)], axis=2)
    return np.ascontiguousarray(a.reshape(KT * 128, 2 * E))


def _chain(prev, cur):
    if prev is not None:
        bass._add_dep_helper(cur.ins, prev.ins, sync=False, reason="order")
    return cur


def build_nc(repeat=1, corr_mode="dr"):
    """corr_mode: 'dr' = full fp8 DoubleRow corrections; 'nodr' = fp8 evac
    ops but no DR matmuls (timing bisect); 'main' = main pass only."""
    nc = bacc.Bacc(None)
    h_ext = nc.declare_dram_parameter("h", [T_CORE, H], F32, isOutput=False)
    w1_ext = nc.declare_dram_parameter("w1", [H, E], F32R, isOutput=False)
    wc_ext = nc.declare_dram_parameter(
        "wc", [KT * 128, 2 * E], F8E5, isOutput=False
    )
    b_ext = nc.declare_dram_parameter("b", [128, E], F32, isOutput=False)
    idx_ext = nc.declare_dram_parameter("idx", [T_CORE, TOP_K], I32, isOutput=True)
    wout_ext = nc.declare_dram_parameter("wout", [T_CORE, TOP_K], F32, isOutput=True)

    # natural views: rows contiguous in DRAM
    h_nat = h_ext[:].rearrange("(tt p) k -> tt p k", p=128)  # [16,128,7168]
    w1_nat = w1_ext[:].rearrange("(kt p) e -> kt p e", p=128)  # [56,128,256]
    wc_nat = wc_ext[:].rearrange("(kt p) e -> kt p e", p=128)  # [56,128,512]

    with tile.TileContext(nc) as tc:
        with (
            tc.tile_pool(name="wpool", bufs=1) as wpool,
            tc.tile_pool(name="hnat", bufs=16) as hnat_pool,
            tc.tile_pool(name="h1T", bufs=4) as h1T_pool,
            tc.tile_pool(name="hc", bufs=4) as hc_pool,
            tc.tile_pool(name="lgT", bufs=2) as lgT_pool,
            tc.tile_pool(name="route", bufs=2) as route_pool,
            tc.tile_pool(name="small", bufs=2) as small_pool,
            tc.tile_pool(name="pst", bufs=2, space="PSUM") as pst_pool,
            tc.tile_pool(name="psg", bufs=4, space="PSUM") as psg_pool,
        ):
            ident = wpool.tile([128, 128], F32, tag="ident")
            make_identity(nc, ident[:])

            bias_sb = wpool.tile([128, E], F32, tag="bias")
            nc.sync.dma_start(out=bias_sb[:], in_=b_ext[:])

            # ---- resident weights: w1 (f32r) + e5m2 DR term tiles
            wT1 = [
                wpool.tile([128, E], F32R, tag=f"w1_{k}", name=f"w1_{k}")
                for k in range(KT)
            ]
            for k in range(KT):
                nc.sync.dma_start(out=wT1[k][:], in_=w1_nat[k])
            wc = [
                wpool.tile([128, 2 * E], F8E5, tag=f"wc_{k}", name=f"wc_{k}")
                for k in range(KT)
            ]
            for k in range(KT):
                nc.sync.dma_start(out=wc[k][:], in_=wc_nat[k])

            # ---- main loop
            prev_stop = {}  # (e, c) -> last mm of previous half (psum chain)
            prev_tr = None  # PE transpose chain across banks
            for rep in range(repeat):
                for half in range(N_HALF):
                    t0 = half * TT_HALF  # first token-tile of half
                    gps = {}
                    for e in range(2):
                        for c in range(2):
                            gps[(e, c)] = psg_pool.tile(
                                [128, 512], F32, tag="psg",
                                name=f"g{rep}_{half}_{e}_{c}",
                            )
                    def do_main(k, h1T):
                        for e in range(2):
                            for c in range(2):
                                mm = nc.tensor.matmul(
                                    gps[(e, c)][:],
                                    wT1[k][:, e * 128 : (e + 1) * 128],
                                    h1T[:, c * 512 : (c + 1) * 512],
                                    start=(k == 0),
                                    stop=False,
                                )
                                if k == 0 and (e, c) in prev_stop:
                                    _chain(prev_stop[(e, c)], mm)

                    def do_corr(k, hc_t, last):
                        # one DR group per k-tile: the DR pair dim carries the
                        # two correction terms: w1s*h2 + w2s*(h1*2^-6)
                        if corr_mode != "dr":
                            if last:  # close the accumulation group
                                for e in range(2):
                                    for c in range(2):
                                        mm = nc.tensor.matmul(
                                            gps[(e, c)][:],
                                            wT1[0][:, e * 128 : (e + 1) * 128],
                                            last_h1T[0][:, c * 512 : (c + 1) * 512],
                                            start=False,
                                            stop=True,
                                        )
                                        prev_stop[(e, c)] = mm
                            return
                        ws3 = wc[k][:].rearrange("p (i e) -> p i e", i=2)
                        hs3 = hc_t[:].rearrange("p (i t) -> p i t", i=2)
                        for e in range(2):
                            for c in range(2):
                                is_last = last and e == 1 and c == 1
                                mm = nc.tensor.matmul(
                                    gps[(e, c)][:],
                                    ws3[:, :, e * 128 : (e + 1) * 128],
                                    hs3[:, :, c * 512 : (c + 1) * 512],
                                    start=False,
                                    stop=last,
                                    perf_mode=mybir.MatmulPerfMode.DoubleRow,
                                )
                                if last:
                                    prev_stop[(e, c)] = mm

                    main_q = []   # (k, h1T) awaiting main matmuls (1-slot lag)
                    corr_q = []   # (k, hc_t) awaiting DR (2-slot lag)
                    last_h1T = [None]
                    for kc in range(KC):
                        hn = []
                        for tt in range(TT_HALF):
                            t = t0 + tt
                            hh = hnat_pool.tile(
                                [128, KPC * 128], F32, tag="h_nat"
                            )
                            c0 = kc * KPC * 128
                            nc.sync.dma_start(
                                out=hh[:],
                                in_=h_nat[t][:, c0 : c0 + KPC * 128],
                            )
                            hn.append(hh)
                        for kk in range(KPC):
                            k = kc * KPC + kk
                            h1T = h1T_pool.tile([128, HALF], F32R, tag="h1T")
                            last_h1T[0] = h1T
                            hc_t = hc_pool.tile([128, 2 * HALF], F8E5, tag="hc")
                            pst = pst_pool.tile(
                                [128, HALF], F32, tag="pst",
                                name=f"p{rep}_{half}_{k}",
                            )
                            for j in range(8):
                                tr = nc.tensor.matmul(
                                    pst[:, j * 128 : (j + 1) * 128],
                                    hn[j][:, kk * 128 : (kk + 1) * 128],
                                    ident[:],
                                    is_transpose=True,
                                    start=(j % 4 == 0),
                                    stop=(j % 4 == 3),
                                )
                                prev_tr = _chain(prev_tr, tr)
                            # hi: ACT rounds psum fp32 -> f32r (one op per k)
                            nc.scalar.copy(h1T[:], pst[:])
                            if corr_mode != "main":
                                # h2 residual straight to e5m2: DVE
                                nc.vector.tensor_tensor(
                                    hc_t[:, :HALF],
                                    pst[:],
                                    h1T[:].bitcast(F32),
                                    mybir.AluOpType.subtract,
                                )
                                # h1 * 2^-6 to e5m2: ACT scale-copy
                                nc.scalar.activation(
                                    hc_t[:, HALF:],
                                    h1T[:].bitcast(F32),
                                    mybir.ActivationFunctionType.Copy,
                                    scale=2.0 ** -6,
                                )
                            corr_q.append((k, hc_t))
                            main_q.append((k, h1T))
                            # trailing issue: main lags 1 slot, DR lags 2
                            if len(main_q) > 1:
                                do_main(*main_q.pop(0))
                            if len(corr_q) > 2:
                                do_corr(*corr_q.pop(0), last=False)
                    do_main(*main_q.pop(0))
                    do_corr(*corr_q.pop(0), last=False)
                    do_corr(*corr_q.pop(0), last=True)

                    # ---- logits^T evacuation (DVE; ACT is the hot engine)
                    lgT = []
                    for e in range(2):
                        lg = lgT_pool.tile([128, HALF], F32, tag="lgT")
                        for c in range(2):
                            nc.vector.tensor_copy(
                                lg[:, c * 512 : (c + 1) * 512], gps[(e, c)][:]
                            )
                        lgT.append(lg)

                    # out-transposes allocate from the psg pool (gps bufs are
                    # free right after the lgT evac) so the pst pool is left
                    # for the NEXT half's input transposes — without this the
                    # next half stalls until routing's sigmoid reads psum
                    for tp in range(TT_HALF // 2):  # 2 t-tiles per tile
                        pso = psg_pool.tile(
                            [128, 512], F32, tag="psg",
                            name=f"o{rep}_{half}_{tp}",
                        )
                        for j in range(4):
                            tt = tp * 2 + j // 2
                            e = j % 2
                            tr = nc.tensor.matmul(
                                pso[:, j * 128 : (j + 1) * 128],
                                lgT[e][:, tt * 128 : (tt + 1) * 128],
                                ident[:],
                                is_transpose=True,
                                start=(j == 0),
                                stop=(j == 3),
                            )
                            prev_tr = _chain(prev_tr, tr)
                        for j in range(2):
                            t = t0 + tp * 2 + j
                            _routing(
                                nc, tc, route_pool, small_pool,
                                pso[:, j * 256 : (j + 1) * 256],
                                bias_sb, idx_ext, wout_ext, t,
                            )

    nc.finalize()
    return nc


def _routing(nc, tc, route_pool, small_pool, logits_ap, bias_sb, idx_ext,
             wout_ext, t):
    """Top-k routing for one 128-token tile.

    Engine split: ACT does sigmoid; Pool (nc.gpsimd) takes wide elementwise
    work (SBUF-only — no PSUM port); DVE keeps the max8/max_index/
    match_replace family plus broadcast/AP-scalar ops Pool can't lower."""
    sc = route_pool.tile([128, E], F32, tag="sc")
    nc.scalar.activation(sc[:], logits_ap, mybir.ActivationFunctionType.Sigmoid)
    scb = route_pool.tile([128, E], F32, tag="scb")
    nc.vector.tensor_add(scb[:], sc[:], bias_sb[:])
    scb3 = scb[:].rearrange("p (g e) -> p g e", e=PER_GROUP)

    gmax = small_pool.tile([128, N_GROUP * 8], F32, tag="gmax")
    for g in range(N_GROUP):
        nc.vector.max(
            gmax[:, g * 8 : g * 8 + 8],
            scb[:, g * PER_GROUP : (g + 1) * PER_GROUP],
        )
    gs = small_pool.tile([128, N_GROUP], F32, tag="gs")
    gm3 = gmax[:].rearrange("p (g k) -> p g k", k=8)
    nc.vector.tensor_add(gs[:], gm3[:, :, 0], gm3[:, :, 1])

    g8 = small_pool.tile([128, 8], F32, tag="g8")
    nc.vector.max(g8[:], gs[:])
    gpen = small_pool.tile([128, N_GROUP], F32, tag="gpen")
    nc.vector.tensor_scalar(
        gpen[:], gs[:], g8[:, 3:4], -1.0,
        mybir.AluOpType.is_ge, mybir.AluOpType.add,
    )
    tmp = route_pool.tile([128, E], F32, tag="tmp")
    tmp3 = tmp[:].rearrange("p (g e) -> p g e", e=PER_GROUP)
    gpen3 = gpen[:, :, None].to_broadcast([128, N_GROUP, PER_GROUP])
    nc.vector.scalar_tensor_tensor(
        tmp3, gpen3, BIG, scb3, mybir.AluOpType.mult, mybir.AluOpType.add,
    )

    v8 = small_pool.tile([128, 8], F32, tag="v8")
    idx8 = small_pool.tile([128, 8], U32, tag="idx8")
    nc.vector.max(v8[:], tmp[:])
    nc.vector.max_index(idx8[:], v8[:], tmp[:])

    mr = route_pool.tile([128, E], F32, tag="mr")
    nc.vector.match_replace(mr[:], v8[:], tmp[:], 2.0 * BIG)
    m01 = route_pool.tile([128, E], F32, tag="m01")
    nc.vector.tensor_scalar(
        m01[:], mr[:], 1.5 * BIG, None, mybir.AluOpType.is_ge
    )
    ssel = route_pool.tile([128, E], F32, tag="ssel")
    nc.vector.tensor_mul(ssel[:], sc[:], m01[:])

    s8 = small_pool.tile([128, 8], F32, tag="s8")
    i8 = small_pool.tile([128, 8], U32, tag="i8")
    nc.vector.max(s8[:], ssel[:])
    nc.vector.max_index(i8[:], s8[:], ssel[:])

    idx8f = small_pool.tile([128, 8], F32, tag="idx8f")
    i8f = small_pool.tile([128, 8], F32, tag="i8f")
    nc.gpsimd.tensor_copy(idx8f[:], idx8[:])
    nc.gpsimd.tensor_copy(i8f[:], i8[:])
    iseq = small_pool.tile([128, 64], F32, tag="iseq")
    iseq3 = iseq[:].rearrange("p (j m) -> p j m", m=8)
    nc.vector.tensor_tensor(
        iseq3,
        idx8f[:, :, None].to_broadcast([128, 8, 8]),
        i8f[:, None, :].to_broadcast([128, 8, 8]),
        mybir.AluOpType.is_equal,
    )
    wsel = small_pool.tile([128, 64], F32, tag="wsel")
    wsel3 = wsel[:].rearrange("p (j m) -> p j m", m=8)
    nc.vector.tensor_tensor(
        wsel3, iseq3, s8[:, None, :].to_broadcast([128, 8, 8]),
        mybir.AluOpType.mult,
    )
    wj = small_pool.tile([128, 8], F32, tag="wj")
    nc.vector.reduce_sum(wj[:], wsel3, axis=mybir.AxisListType.X)

    sum8 = small_pool.tile([128, 1], F32, tag="sum8")
    nc.vector.reduce_sum(sum8[:], wj[:], axis=mybir.AxisListType.X)
    seps = small_pool.tile([128, 1], F32, tag="seps")
    nc.vector.tensor_scalar_add(seps[:], sum8[:], 1.0e-20)
    rec = small_pool.tile([128, 1], F32, tag="rec")
    nc.vector.reciprocal(rec[:], seps[:])
    wout = small_pool.tile([128, 8], F32, tag="wout")
    nc.vector.tensor_scalar(
        wout[:], wj[:], rec[:, 0:1], ROUTED_SCALING,
        mybir.AluOpType.mult, mybir.AluOpType.mult,
    )

    nc.sync.dma_start(
        out=idx_ext[t * 128 : (t + 1) * 128, :], in_=idx8[:].bitcast(I32)
    )
    nc.sync.dma_start(
        out=wout_ext[t * 128 : (t + 1) * 128, :], in_=wout[:]
    )


_NC_CACHE = None


def kernel(hidden_states, weight, e_score_correction_bias):
    global _NC_CACHE, LAST_EXEC_NS
    h = np.ascontiguousarray(
        np.asarray(hidden_states, dtype=np.float32)
    ).reshape(T_FULL, H)
    wT = np.ascontiguousarray(np.asarray(weight, dtype=np.float32).T)
    w1 = rnd11(wT)
    wc = dr_pack_terms(_e5m2(w1), _e5m2((wT - w1) * 2.0 ** 6))
    b = np.asarray(e_score_correction_bias, dtype=np.float32)
    b_bcast = np.ascontiguousarray(np.broadcast_to(b[None, :], (128, E)))

    if _NC_CACHE is None:
        _NC_CACHE = build_nc()
    nc = _NC_CACHE

    in_maps = [
        {"h": h[c * T_CORE : (c + 1) * T_CORE], "w1": w1,
         "wc": wc, "b": b_bcast}
        for c in range(N_CORES)
    ]
    res = run_bass_kernel_spmd(nc, in_maps, core_ids=list(range(N_CORES)))
    LAST_EXEC_NS = res.exec_time_ns

    idx = np.concatenate([res.results[c]["idx"] for c in range(N_CORES)], axis=0)
    wout = np.concatenate([res.results[c]["wout"] for c in range(N_CORES)], axis=0)
    return idx.astype(np.int32), wout.astype(np.float32)
